# revision 9
# baseline (speedup 1.0000x reference)
"""Multi-head attention (B=4, S=2048, D=1024, H=16) on 8 trn2 NeuronCores.

Sharding: core = (batch b, head-group g) with b = core//2, g = core%2.
Each core handles one batch and 8 heads (512 of the 1024 d_model dims):
  - host pre-transposes query/key/value[b] -> [1024, 2048] so the device
    never transposes activations
  - device computes Q^T, K^T (head dims on partitions) and V (natural),
    attention with *transposed* scores S^T = K_h @ Q_h^T so softmax's
    denominator comes out of the PV matmul via a ones-column appended to V
  - output projection vs Wo[g*512:(g+1)*512, :] gives a partial [2048,1024]
  - host sums the two group partials per batch and adds bv@Wo + bo
All matmuls run as float32r (fp32 data, fp22 multiply, fp32 accumulate).
"""

import numpy as np
from contextlib import ExitStack

B = 4
S = 2048
D = 1024
H = 16
DK = 64
NCORES = 8
GH = 8          # heads per core (group)
GD = GH * DK    # 512 head dims per core
NCH = GD // 128  # 4 chunks of 128 output dims
KT = S // 128    # 16 key tiles
QC = 1024        # q chunk width for attention
NQC = S // QC    # 2
SC = 256         # s chunk width for projections
NSC = S // SC    # 8
DMT = D // 128   # 8 d_model tiles

_CACHE = {}


def _build_program():
    import concourse.mybir as mybir
    import concourse.tile as tile
    from concourse import bacc

    f32 = mybir.dt.float32
    f32r = mybir.dt.float32r

    nc = bacc.Bacc("TRN2", target_bir_lowering=False, debug=False,
                   num_devices=NCORES)

    xqT = nc.dram_tensor("xqT", [D, S], f32r, kind="ExternalInput").ap()
    xkT = nc.dram_tensor("xkT", [D, S], f32r, kind="ExternalInput").ap()
    xvT = nc.dram_tensor("xvT", [D, S], f32r, kind="ExternalInput").ap()
    wq = nc.dram_tensor("wq", [D, GD], f32r, kind="ExternalInput").ap()
    wk = nc.dram_tensor("wk", [D, GD], f32r, kind="ExternalInput").ap()
    wv = nc.dram_tensor("wv", [D, GD], f32r, kind="ExternalInput").ap()
    wo = nc.dram_tensor("wo", [GD, D], f32r, kind="ExternalInput").ap()
    bq = nc.dram_tensor("bq", [GD], f32, kind="ExternalInput").ap()
    bk = nc.dram_tensor("bk", [GD], f32, kind="ExternalInput").ap()
    out = nc.dram_tensor("out", [S, D], f32, kind="ExternalOutput").ap()
    import os as _os
    dbg = _os.environ.get("DEBUG_DUMPS", "0") == "1"
    if dbg:
        d_qt = nc.dram_tensor("d_qt", [128, S], f32, kind="ExternalOutput").ap()
        d_kt = nc.dram_tensor("d_kt", [128, S], f32, kind="ExternalOutput").ap()
        d_v = nc.dram_tensor("d_v", [128, GH * 65], f32, kind="ExternalOutput").ap()
        d_pt = nc.dram_tensor("d_pt", [128, QC], f32, kind="ExternalOutput").ap()
        d_pv = nc.dram_tensor("d_pv", [65, QC], f32, kind="ExternalOutput").ap()
        d_zr = nc.dram_tensor("d_zr", [1, QC], f32, kind="ExternalOutput").ap()
        d_rb = nc.dram_tensor("d_rb", [DK, QC], f32, kind="ExternalOutput").ap()
        d_ot = nc.dram_tensor("d_ot", [128, S], f32, kind="ExternalOutput").ap()

    Exp = mybir.ActivationFunctionType.Exp

    with tile.TileContext(nc) as tc, ExitStack() as ctx:
        # ---- pools (slots are statically reserved per tag) ----
        p_qt = ctx.enter_context(tc.tile_pool(name="qt", bufs=NCH))    # 32K
        p_kt = ctx.enter_context(tc.tile_pool(name="kt", bufs=NCH))    # 32K
        p_v = ctx.enter_context(tc.tile_pool(name="v", bufs=KT))       # ~35K
        p_ot = ctx.enter_context(tc.tile_pool(name="ot", bufs=NCH))    # 32K
        p_wvo = ctx.enter_context(tc.tile_pool(name="wvo", bufs=1))    # 16K
        p_wc = ctx.enter_context(tc.tile_pool(name="wc", bufs=2))      # 16K
        p_bias = ctx.enter_context(tc.tile_pool(name="bias", bufs=1))
        p_xs = ctx.enter_context(tc.tile_pool(name="xs", bufs=2))      # 16K
        p_pt = ctx.enter_context(tc.tile_pool(name="pt", bufs=2))      # 8K
        p_zr = ctx.enter_context(tc.tile_pool(name="zr", bufs=1))      # 4K
        p_rb = ctx.enter_context(tc.tile_pool(name="rb", bufs=2))      # 8K
        p_st = ctx.enter_context(tc.tile_pool(name="st", bufs=2))      # 8K
        # PSUM: 2-bank slots x 2 bufs x 2 pools = all 8 banks
        p_ps = ctx.enter_context(tc.tile_pool(name="ps", bufs=2, space="PSUM"))
        p_pv = ctx.enter_context(tc.tile_pool(name="pv", bufs=2, space="PSUM"))

        # ---- biases ----
        bq_sb = p_bias.tile([128, NCH], f32, tag="bq")
        nc.sync.dma_start(out=bq_sb[:], in_=bq.rearrange("(a p) -> p a", p=128))
        bk_sb = p_bias.tile([128, NCH], f32, tag="bk")
        nc.sync.dma_start(out=bk_sb[:], in_=bk.rearrange("(a p) -> p a", p=128))

        # ---- V projection: V_sb[st] = [128 s, GH, 65] (col 64 = ones) ----
        ones_sb = p_bias.tile([128, 1], f32, tag="ones")
        nc.vector.memset(ones_sb[:], 1.0)
        wv_sb = p_wvo.tile([128, DMT, GD], f32r, tag="wvo", name="wv_sb")
        nc.sync.dma_start(out=wv_sb[:], in_=wv.rearrange("(a p) d -> p a d", p=128))
        v_sb = []
        for st in range(KT):
            xv_t = p_xs.tile([128, DMT, SC], f32r, tag="xs", name=f"xv{st}")
            nc.sync.dma_start(
                out=xv_t[:, :, 0:128],
                in_=xvT[:, st * 128:(st + 1) * 128].rearrange(
                    "(a p) s -> p a s", p=128),
            )
            ps = p_ps.tile([128, 1024], f32, tag="ps", name=f"psv{st}")
            for a in range(DMT):
                nc.tensor.matmul(
                    out=ps[:, 0:GD],
                    lhsT=xv_t[:, a, 0:128],
                    rhs=wv_sb[:, a, :],
                    start=(a == 0), stop=(a == DMT - 1),
                )
            vt = p_v.tile([128, GH, 65], f32r, tag="v", name=f"v{st}")
            nc.vector.tensor_copy(
                out=vt[:, :, 0:DK],
                in_=ps[:, 0:GD].rearrange("p (h d) -> p h d", h=GH),
            )
            nc.vector.tensor_copy(
                out=vt[:, :, DK:65],
                in_=ones_sb.unsqueeze(1).broadcast_to([128, GH, 1]))
            v_sb.append(vt)

        qt_sb = [None] * NCH
        kt_sb = [None] * NCH
        ot_sb = [None] * NCH

        def proj_chunk(c):
            """Q^T and K^T chunk c: [128 dout, S]."""
            qt_sb[c] = p_qt.tile([128, S], f32r, tag="qt", name=f"qt{c}")
            kt_sb[c] = p_kt.tile([128, S], f32r, tag="kt", name=f"kt{c}")
            wq_c = p_wc.tile([128, DMT, 128], f32r, tag="wqc", name=f"wq{c}")
            nc.sync.dma_start(
                out=wq_c[:],
                in_=wq[:, c * 128:(c + 1) * 128].rearrange(
                    "(a p) d -> p a d", p=128))
            wk_c = p_wc.tile([128, DMT, 128], f32r, tag="wkc", name=f"wk{c}")
            nc.sync.dma_start(
                out=wk_c[:],
                in_=wk[:, c * 128:(c + 1) * 128].rearrange(
                    "(a p) d -> p a d", p=128))
            for src, wsb, bsb, dst, nm in (
                (xqT, wq_c, bq_sb, qt_sb[c], "q"),
                (xkT, wk_c, bk_sb, kt_sb[c], "k"),
            ):
                for sc in range(NSC):
                    xs = p_xs.tile([128, DMT, SC], f32r, tag="xs",
                                   name=f"xs{nm}{c}_{sc}")
                    nc.sync.dma_start(
                        out=xs[:],
                        in_=src[:, sc * SC:(sc + 1) * SC].rearrange(
                            "(a p) s -> p a s", p=128),
                    )
                    ps = p_ps.tile([128, 1024], f32, tag="ps",
                                   name=f"psp{nm}{c}_{sc}")
                    for a in range(DMT):
                        nc.tensor.matmul(
                            out=ps[:, 0:SC],
                            lhsT=wsb[:, a, :],
                            rhs=xs[:, a, :],
                            start=(a == 0), stop=(a == DMT - 1),
                        )
                    nc.vector.tensor_scalar_add(
                        out=dst[:, sc * SC:(sc + 1) * SC],
                        in0=ps[:, 0:SC],
                        scalar1=bsb[:, c:c + 1],
                    )

        def attention_pair(c):
            """Heads 2c, 2c+1 -> normalized O^T chunk c [128 dout, S]."""
            ot_sb[c] = p_ot.tile([128, S], f32r, tag="ot", name=f"ot{c}")
            for qc in range(NQC):
                pvs = [p_pv.tile([65, QC], f32, tag="pv",
                                 name=f"pv{c}_{qc}_{h}") for h in range(2)]
                for kt_i in range(KT):
                    for hh in range(2):
                        hg = 2 * c + hh
                        ps = p_ps.tile([128, QC], f32, tag="ps",
                                       name=f"pss{c}_{qc}_{kt_i}_{hh}")
                        for half in range(QC // 512):
                            q0 = qc * QC + half * 512
                            nc.tensor.matmul(
                                out=ps[:, half * 512:(half + 1) * 512],
                                lhsT=kt_sb[c][hh * DK:(hh + 1) * DK,
                                              kt_i * 128:(kt_i + 1) * 128
                                              ],
                                rhs=qt_sb[c][hh * DK:(hh + 1) * DK,
                                             q0:q0 + 512],
                                start=True, stop=True,
                            )
                        pt = p_pt.tile([128, QC], f32r, tag="pt",
                                       name=f"pt{c}_{qc}_{kt_i}_{hh}")
                        nc.scalar.activation(pt[:], ps[:], Exp,
                                             bias=0.0, scale=0.125)
                        if dbg and c == 0 and qc == 0 and kt_i == 0 and hh == 0:
                            nc.sync.dma_start(out=d_pt[:], in_=pt[:].bitcast(f32))
                        for half in range(QC // 512):
                            nc.tensor.matmul(
                                out=pvs[hh][:, half * 512:(half + 1) * 512],
                                lhsT=v_sb[kt_i][:, hg, :],
                                rhs=pt[:, half * 512:(half + 1) * 512
                                       ],
                                start=(kt_i == 0), stop=(kt_i == KT - 1),
                            )
                # normalize by Z (= row 64 of pv psum) and store into O^T
                for hh in range(2):
                    zr = p_zr.tile([1, QC], f32, tag="zr",
                                   name=f"zr{c}_{qc}_{hh}")
                    nc.vector.reciprocal(out=zr[:], in_=pvs[hh][DK:DK + 1, :])
                    rb = p_rb.tile([DK, QC], f32, tag="rb",
                                   name=f"rb{c}_{qc}_{hh}")
                    nc.gpsimd.partition_broadcast(rb[:], zr[:], channels=DK)
                    if dbg and c == 0 and qc == 0 and hh == 0:
                        dpv_sb = p_st.tile([65, QC], f32, tag="st",
                                           name="dpv_sb")
                        nc.vector.tensor_copy(out=dpv_sb[:], in_=pvs[hh][:])
                        nc.sync.dma_start(out=d_pv[:], in_=dpv_sb[:])
                        nc.sync.dma_start(out=d_zr[:], in_=zr[:])
                        nc.sync.dma_start(out=d_rb[:], in_=rb[:])
                    if hh == 0:
                        nc.vector.tensor_mul(
                            out=ot_sb[c][0:DK, qc * QC:(qc + 1) * QC],
                            in0=pvs[hh][0:DK, :], in1=rb[:])
                    else:
                        tmp = p_rb.tile([DK, QC], f32r, tag="rb",
                                        name=f"tmp{c}_{qc}")
                        nc.vector.tensor_mul(out=tmp[:], in0=pvs[hh][0:DK, :],
                                             in1=rb[:])
                        nc.sync.dma_start(
                            out=ot_sb[c][DK:128, qc * QC:(qc + 1) * QC],
                            in_=tmp[:])

        # ---- emit: projections interleaved with attention ----
        for c in range(NCH):
            proj_chunk(c)
            attention_pair(c)

        if dbg:
            nc.sync.dma_start(out=d_qt[:], in_=qt_sb[0][:].bitcast(f32))
            nc.sync.dma_start(out=d_kt[:], in_=kt_sb[0][:].bitcast(f32))
            nc.sync.dma_start(out=d_v[:],
                              in_=v_sb[0][:].bitcast(f32).rearrange("p a b -> p (a b)"))
            nc.sync.dma_start(out=d_ot[:], in_=ot_sb[0][:].bitcast(f32))

        # ---- output projection ----
        wo_sb = p_wvo.tile([128, NCH, D], f32r, tag="wvo", name="wo_sb")
        nc.sync.dma_start(out=wo_sb[:], in_=wo.rearrange("(a p) n -> p a n", p=128))
        for qt_i in range(KT):
            ps = p_ps.tile([128, 1024], f32, tag="ps", name=f"pso{qt_i}")
            for c in range(NCH):
                for half in range(2):
                    nc.tensor.matmul(
                        out=ps[:, half * 512:(half + 1) * 512],
                        lhsT=ot_sb[c][:, qt_i * 128:(qt_i + 1) * 128
                                      ],
                        rhs=wo_sb[:, c, half * 512:(half + 1) * 512
                                  ],
                        start=(c == 0), stop=(c == NCH - 1),
                    )
            st = p_st.tile([128, D], f32, tag="st", name=f"st{qt_i}")
            nc.vector.tensor_copy(out=st[:], in_=ps[:])
            nc.sync.dma_start(out=out[qt_i * 128:(qt_i + 1) * 128, :], in_=st[:])

    nc.compile()
    return nc


def get_program():
    if "nc" not in _CACHE:
        _CACHE["nc"] = _build_program()
    return _CACHE["nc"]


def make_in_maps(inputs):
    q = np.asarray(inputs["query"], np.float32)
    k = np.asarray(inputs["key"], np.float32)
    v = np.asarray(inputs["value"], np.float32)
    Wq = np.asarray(inputs["Wq"], np.float32)
    Wk = np.asarray(inputs["Wk"], np.float32)
    Wv = np.asarray(inputs["Wv"], np.float32)
    Wo = np.asarray(inputs["Wo"], np.float32)
    bq = np.asarray(inputs["bq"], np.float32)
    bk = np.asarray(inputs["bk"], np.float32)
    in_maps = []
    for core in range(NCORES):
        b, g = core // 2, core % 2
        sl = slice(g * GD, (g + 1) * GD)
        in_maps.append({
            "xqT": np.ascontiguousarray(q[b].T),
            "xkT": np.ascontiguousarray(k[b].T),
            "xvT": np.ascontiguousarray(v[b].T),
            "wq": np.ascontiguousarray(Wq[:, sl]),
            "wk": np.ascontiguousarray(Wk[:, sl]),
            "wv": np.ascontiguousarray(Wv[:, sl]),
            "wo": np.ascontiguousarray(Wo[sl, :]),
            "bq": np.ascontiguousarray(bq[sl]),
            "bk": np.ascontiguousarray(bk[sl]),
        })
    return in_maps


def combine_outputs(results, inputs):
    Wo = np.asarray(inputs["Wo"], np.float32)
    bv = np.asarray(inputs["bv"], np.float32)
    bo = np.asarray(inputs["bo"], np.float32)
    out = np.empty((B, S, D), np.float32)
    for b in range(B):
        out[b] = results[2 * b]["out"] + results[2 * b + 1]["out"]
    out += bv @ Wo + bo
    return out


def kernel(**inputs):
    from concourse.bass_utils import run_bass_kernel_spmd
    nc = get_program()
    in_maps = make_in_maps(inputs)
    res = run_bass_kernel_spmd(nc, in_maps, list(range(NCORES)))
    return combine_outputs(res.results, inputs)
